# revision 15
# baseline (speedup 1.0000x reference)
"""Multi-head attention (B=2, S=2048, E=1024, H=16) on 8 Trainium2 NeuronCores.

Sharding: core c handles batch b=c//4 and head group g=c%4 (4 heads each).
hidden_states[b] is replicated to the 4 cores of batch b (pre-transposed and
cast to bf16 on host so the contraction dim E lands on SBUF partitions with
plain contiguous DMAs). Each core computes q/k/v projections for its heads,
transposed-layout attention, and a partial output projection over its 256
E-dims. The host sums the 4 partials per batch and adds bo.

Schedule: a single software-pipelined stream. Projections (k/q/v chains) are
injected into the first s-tile's attention iterations so the exp stream on the
Scalar engine starts ~8us in and the PE never idles long enough for the HAM
clock gate to re-throttle. The softmax denominator is fused into the ctx
matmul: head A's stationary is [vA(64) | ones | 0*63] (ctx at psum partitions
0:64, denom at 64), head B's is [ones | 0*63 | vB(64)] (denom at partition 0,
ctx at 64:128) so all downstream DVE ops are partition-aligned and the output
projection keeps C=128 contraction. Normalization: denominators are copied to
fp16, broadcast over 64 partitions with a tiny ones-matmul, reciprocal'd once
per [128,512] tile, multiplied into cn, bias-added.

Bias handling: softmax over t is invariant to per-query constants, so the
k-bias drops out and the q-bias folds into q. The v-bias is a post-softmax
additive constant (softmax rows sum to 1). bo is added on host.
"""

import sys

if "/opt/trn_rl_repo" not in sys.path:
    sys.path.insert(0, "/opt/trn_rl_repo")

import numpy as np
import ml_dtypes

import concourse.bass as bass
import concourse.tile as tile
from concourse import mybir
from concourse.bass_utils import run_bass_kernel_spmd
from concourse.vector_clock import ScopedClock

B, S, E, H = 2, 2048, 1024, 16
DH = E // H  # 64
N_CORES = 8
HEADS_PER_CORE = 4  # 2 pairs
EL = HEADS_PER_CORE * DH  # 256 local E-dims per core

F32 = mybir.dt.float32
BF16 = mybir.dt.bfloat16
FP16 = mybir.dt.float16
FP8 = mybir.dt.float8e4
BF16_NP = ml_dtypes.bfloat16

ST = 512  # s_tile width
N_ST = S // ST  # 4
N_TC = S // 128  # 16 t-chunks
N_EC = E // 128  # 8 e-chunks
LAG = 4  # ctx matmuls trail scores/exp by this many iterations


def _patch_tail_drain():
    """walrus CoreV3 setupSyncWait allows only 1 sem wait on an SP Drain; Tile's
    kernel-tail drain carries one wait per live processor. Split the waits
    across consecutive drains (mutating via nc.inst_map, whose objects are what
    to_json_bytes serializes)."""
    if getattr(tile.TileContext, "_drain_patched", False):
        return

    def _drain_and_barrier(self, tick_clock, wait_clock):
        nc = self.nc
        drain_inst = nc.sync.drain()
        wait_clock.add_sem_waits(
            drain_inst.ins, ScopedClock({None: tick_clock.global_clock})
        )
        inst = nc.inst_map[drain_inst.ins.name]
        w = list(inst.sync_info.on_wait) if inst.sync_info else []
        if len(w) > 1:
            si = inst.sync_info
            si.on_wait = w[:1]
            inst.sync_info = si
            for i in range(1, len(w)):
                d2 = nc.sync.drain()
                i2 = nc.inst_map[d2.ins.name]
                si2 = i2.sync_info or mybir.SyncInfo(on_wait=[], on_update=[])
                si2.on_wait = [w[i]]
                i2.sync_info = si2
        nc.all_engine_barrier()
        assert self.sems is not None
        popped = nc._tile_sem_poison_stack.pop()
        assert popped is self._sem_poison
        nc.clear_and_free_semaphores(list(self.sems.allocated().values()))
        nc.all_engine_barrier()

    tile.TileContext._drain_and_barrier = _drain_and_barrier
    tile.TileContext._drain_patched = True


def _split_multi_waits(nc):
    """The walrus build in this environment accepts only ONE sem-wait command
    per instruction, but Tile's wait-assignment attaches several. Hoist excess
    waits onto dedicated same-engine no-op carrier instructions inserted
    immediately before the owner (same engine-stream position, identical
    semantics)."""
    f = nc.m.functions[0]
    blocks = list(f.blocks)
    carriers: dict[str, list] = {}
    created = set()
    for blk in blocks:
        for inst in blk.instructions:
            if inst.sync_info and len(inst.sync_info.on_wait) > 1:
                w = list(inst.sync_info.on_wait)
                cs = []
                for wx in w[:-1]:
                    # engine nop() appends to nc.cur_bb; it is re-homed below
                    nop = nc.engines[inst.engine].nop(nofuse=True).ins
                    nop.sync_info = mybir.SyncInfo(on_wait=[wx], on_update=[])
                    cs.append(nop)
                    created.add(nop.name)
                si = inst.sync_info
                si.on_wait = [w[-1]]
                inst.sync_info = si
                carriers[inst.name] = cs
    if not carriers:
        return
    for blk in blocks:
        rebuilt = []
        for i in blk.instructions:
            if i.name in created:
                continue
            rebuilt.extend(carriers.get(i.name, ()))
            rebuilt.append(i)
        blk.instructions = rebuilt


def build_bass():
    """Build the per-core Bass program (identical on all 8 cores)."""
    _patch_tail_drain()
    nc = bass.Bass("TRN2", target_bir_lowering=False, debug=False)

    xt_d = nc.dram_tensor("xt", [E * S], BF16, kind="ExternalInput").ap()
    wq_d = nc.dram_tensor("wq", [E * EL], BF16, kind="ExternalInput").ap()
    wk_d = nc.dram_tensor("wk", [E * EL], BF16, kind="ExternalInput").ap()
    wv_d = nc.dram_tensor("wv", [E * EL], BF16, kind="ExternalInput").ap()
    wo_d = nc.dram_tensor("wo", [EL * E], BF16, kind="ExternalInput").ap()
    bq_d = nc.dram_tensor("bq2", [128, 2], F32, kind="ExternalInput").ap()
    bv_d = nc.dram_tensor("bv2", [128, 2], F32, kind="ExternalInput").ap()
    out_d = nc.dram_tensor("out", [S, E], BF16, kind="ExternalOutput").ap()

    EXP = mybir.ActivationFunctionType.Exp
    ADD = mybir.AluOpType.add
    MULT = mybir.AluOpType.mult

    with tile.TileContext(nc) as tc:
        with (
            tc.tile_pool(name="const", bufs=1) as const_pool,
            tc.tile_pool(name="xw", bufs=1) as xw_pool,
            tc.tile_pool(name="qkv", bufs=1) as qkv_pool,
            tc.tile_pool(name="exs", bufs=5) as ex_pool,
            tc.tile_pool(name="cns", bufs=3) as cn_pool,
            tc.tile_pool(name="rb32", bufs=2) as rb32_pool,
            tc.tile_pool(name="rbh", bufs=2) as rbh_pool,
            tc.tile_pool(name="dh", bufs=4) as dh_pool,
            tc.tile_pool(name="outs", bufs=3) as out_pool,
            tc.tile_pool(name="psa", bufs=2, space="PSUM") as psa,
            tc.tile_pool(name="psc", bufs=4, space="PSUM") as psc,
        ):
            # ---- constants and weights
            ones1 = const_pool.tile([1, 64], FP16)
            nc.vector.memset(ones1[:], 1.0)
            # Full-array dummy operands: HAM gauges PE *array activity*, so
            # warmers must light up all 128x128 cells (a [1,64] ones matmul
            # reads as idle and the clock gate stays at 1.2GHz).
            dmy_w = const_pool.tile([128, 128], BF16, name="dmy_w")
            nc.vector.memset(dmy_w[:], 0.0)
            dmy_x = const_pool.tile([128, ST], BF16, name="dmy_x")
            nc.vector.memset(dmy_x[:], 0.0)

            bq_sb = const_pool.tile([128, 2], F32)
            nc.sync.dma_start(bq_sb[:], bq_d[:])
            bv_sb = const_pool.tile([128, 2], F32)
            nc.sync.dma_start(bv_sb[:], bv_d[:])

            # weights arrive host-pre-transposed to [128, o, d] so every DMA
            # is contiguous 4KB-per-partition; wk/wq + the first xt quarter are
            # emitted first so the k/q chains start within a few us.
            # per-ec interleaved wk/xt-q0 transfers: the k(0,0) chain's ec-th
            # matmul only depends on the ec-th slices (subtile deps), so the
            # PE starts ~2us after the first pair lands instead of waiting for
            # every input
            wk_sb = xw_pool.tile([128, N_EC, EL], BF16)
            wq_sb = xw_pool.tile([128, N_EC, EL], BF16)
            wv_sb = xw_pool.tile([128, N_EC, EL], BF16)
            xt_sb = xw_pool.tile([128, N_EC, S], BF16)
            wk_r = wk_d.rearrange("(p o d) -> p o d", p=128, o=N_EC)
            wq_r = wq_d.rearrange("(p o d) -> p o d", p=128, o=N_EC)
            wv_r = wv_d.rearrange("(p o d) -> p o d", p=128, o=N_EC)
            xt_q = xt_d.rearrange("(p q o s) -> p q o s", p=128, q=4, o=N_EC)
            for h in range(2):
                e0, e1 = 4 * h, 4 * (h + 1)
                nc.sync.dma_start(wk_sb[:, e0:e1, :], wk_r[:, e0:e1])
                nc.sync.dma_start(xt_sb[:, e0:e1, 0:512], xt_q[:, 0, e0:e1])
            nc.sync.dma_start(wq_sb[:], wq_r)
            nc.sync.dma_start(wv_sb[:], wv_r)
            for q4 in range(1, 4):
                nc.sync.dma_start(
                    xt_sb[:, :, 512 * q4 : 512 * (q4 + 1)], xt_q[:, q4]
                )
            wo_sb = xw_pool.tile([128, 2, E], BF16)
            nc.sync.dma_start(wo_sb[:], wo_d.rearrange("(p o n) -> p o n", p=128, o=2))

            # ---- persistent SBUF tensors
            qT = [qkv_pool.tile([128, S], BF16, name=f"qT{p}") for p in range(2)]
            kT = [qkv_pool.tile([128, S], BF16, name=f"kT{p}") for p in range(2)]
            # v65: per (t-chunk, head) a 128-col stationary.
            #   even head (A): [v(64) | ones | 0*63]  -> ctx @ psum 0:64, den @ 64
            #   odd head (B):  [ones | 0*63 | v(64)]  -> den @ psum 0, ctx @ 64:128
            v65 = qkv_pool.tile([128, N_TC, 4, 128], FP8)
            nc.vector.memset(v65[:, :, 0::2, 64:65], 1.0)
            nc.vector.memset(v65[:, :, 0::2, 65:128], 0.0)
            nc.vector.memset(v65[:, :, 1::2, 0:1], 1.0)
            nc.vector.memset(v65[:, :, 1::2, 1:64], 0.0)

            cn = {}  # (st, p) -> cn tile

            # ---------------- emission closures ----------------
            def k_chain(p, kt):
                ps = psa.tile([128, 2 * ST], F32, tag="a", name="ps_k")
                for ec in range(N_EC):
                    nc.tensor.matmul(
                        ps[:, :ST],
                        wk_sb[:, ec, 128 * p : 128 * (p + 1)],
                        xt_sb[:, ec, ST * kt : ST * (kt + 1)],
                        start=(ec == 0),
                        stop=(ec == N_EC - 1),
                    )
                nc.vector.tensor_copy(kT[p][:, ST * kt : ST * (kt + 1)], ps[:, :ST])

            def q_chain(p, st):
                ps = psa.tile([128, 2 * ST], F32, tag="a", name="ps_q")
                for ec in range(N_EC):
                    nc.tensor.matmul(
                        ps[:, :ST],
                        wq_sb[:, ec, 128 * p : 128 * (p + 1)],
                        xt_sb[:, ec, ST * st : ST * (st + 1)],
                        start=(ec == 0),
                        stop=(ec == N_EC - 1),
                    )
                nc.vector.tensor_scalar(
                    qT[p][:, ST * st : ST * (st + 1)],
                    ps[:, :ST],
                    bq_sb[:, p : p + 1],
                    None,
                    ADD,
                )

            def v_chain(tt):
                ps = psa.tile([128, 2 * ST], F32, tag="a", name="ps_v")
                for ec in range(N_EC):
                    nc.tensor.matmul(
                        ps[:, :EL],
                        xt_sb[:, ec, 128 * tt : 128 * (tt + 1)],
                        wv_sb[:, ec, :],
                        start=(ec == 0),
                        stop=(ec == N_EC - 1),
                    )
                vsrc = ps[:, :EL].rearrange("p (h d) -> p h d", h=4)
                # even heads -> cols 0:64, odd heads -> cols 64:128
                nc.vector.tensor_copy(v65[:, tt, 0::2, 0:64], vsrc[:, 0::2, :])
                nc.vector.tensor_copy(v65[:, tt, 1::2, 64:128], vsrc[:, 1::2, :])

            ctx_ps = {}  # (st, p) -> (ctxA tile, ctxB tile)
            ex_tiles = {}  # (st, p, tc) -> ex tile (deleted after use)

            def scores(st, p, tcc):
                sc = psa.tile([128, 2 * ST], F32, tag="a", name="sc")
                nc.tensor.matmul(
                    sc[:, :ST],
                    kT[p][0:64, 128 * tcc : 128 * (tcc + 1)],
                    qT[p][0:64, ST * st : ST * (st + 1)],
                    start=True,
                    stop=True,
                )
                nc.tensor.matmul(
                    sc[:, ST:],
                    kT[p][64:128, 128 * tcc : 128 * (tcc + 1)],
                    qT[p][64:128, ST * st : ST * (st + 1)],
                    start=True,
                    stop=True,
                )
                return sc

            def exp_emit(st, p, tcc, sc):
                # fp8 softmax weights, written pair-interleaved for DoubleRow
                if tcc % 2 == 0:
                    ex = ex_pool.tile([128, 2, 2 * ST], FP8, name="ex")
                    ex_tiles[(st, p, tcc // 2)] = ex
                else:
                    ex = ex_tiles[(st, p, tcc // 2)]
                nc.scalar.activation(ex[:, tcc % 2, :], sc[:], EXP, scale=0.125)

            def ctx_dr(st, p, tp):
                # one fp8 DoubleRow matmul per head covers two t-chunks
                if (st, p) not in ctx_ps:
                    a = psc.tile([128, ST], F32, tag="c", name="ctxA")
                    b = psc.tile([128, ST], F32, tag="c", name="ctxB")
                    ctx_ps[(st, p)] = (a, b)
                a, b = ctx_ps[(st, p)]
                ex = ex_tiles.pop((st, p, tp))
                first, last = tp == 0, tp == N_TC // 2 - 1
                nc.tensor.matmul(
                    a[:],
                    v65[:, 2 * tp : 2 * tp + 2, 2 * p, :],
                    ex[:, :, :ST],
                    start=first,
                    stop=last,
                    perf_mode=mybir.MatmulPerfMode.DoubleRow,
                )
                nc.tensor.matmul(
                    b[:],
                    v65[:, 2 * tp : 2 * tp + 2, 2 * p + 1, :],
                    ex[:, :, ST:],
                    start=first,
                    stop=last,
                    perf_mode=mybir.MatmulPerfMode.DoubleRow,
                )

            def den_copies(st, p):
                a, b = ctx_ps[(st, p)]
                dAh = dh_pool.tile([1, ST], FP16, tag="dh", name="dAh")
                nc.vector.tensor_copy(dAh[:], a[64:65, :])
                dBh = dh_pool.tile([1, ST], FP16, tag="dh", name="dBh")
                nc.vector.tensor_copy(dBh[:], b[0:1, :])
                ctx_ps[(st, p)] = (a, b, dAh, dBh)

            def norm(st, p):
                a, b, dAh, dBh = ctx_ps.pop((st, p))
                rbp = psa.tile([128, 2 * ST], F32, tag="a", name="rbp")
                nc.tensor.matmul(
                    rbp[0:64, :ST], ones1[:], dAh[:], start=True, stop=True
                )
                nc.tensor.matmul(
                    rbp[64:128, :ST], ones1[:], dBh[:], start=True, stop=True
                )
                rb32 = rb32_pool.tile([128, ST], F32, name="rb32")
                nc.vector.tensor_copy(rb32[:], rbp[:, :ST])
                rbh = rbh_pool.tile([128, ST], FP16, name="rbh")
                with nc.allow_low_precision(reason="fp16 recip of softmax denom"):
                    nc.vector.reciprocal(rbh[:], rb32[:])
                c = cn_pool.tile([128, ST], BF16, name="cn")
                nc.vector.tensor_tensor(c[0:64, :], a[0:64, :], rbh[0:64, :], MULT)
                nc.vector.tensor_tensor(
                    c[64:128, :], b[64:128, :], rbh[64:128, :], MULT
                )
                nc.vector.tensor_scalar(c[:], c[:], bv_sb[:, p : p + 1], None, ADD)
                cn[(st, p)] = c

            def outproj(st, ss):
                # full-E projection of one 128-row block: 4 F=512 matmuls into
                # the two bank-halves of a single ring slot (one alloc, so the
                # scores ring keeps its lookahead), one copy, one DMA
                ps = psa.tile([128, 2 * ST], F32, tag="a", name="ps_o")
                for nt in range(2):
                    for p in range(2):
                        nc.tensor.matmul(
                            ps[:, ST * nt : ST * (nt + 1)],
                            cn[(st, p)][:, 128 * ss : 128 * (ss + 1)],
                            wo_sb[:, p, ST * nt : ST * (nt + 1)],
                            start=(p == 0),
                            stop=(p == 1),
                        )
                ob = out_pool.tile([128, 2 * ST], BF16, name="ob")
                nc.vector.tensor_copy(ob[:], ps[:])
                srow = ST * st + 128 * ss
                nc.sync.dma_start(out_d[srow : srow + 128, :], ob[:])

            # ---------------- schedule ----------------
            # Injections per loop (st, p), keyed by iteration index.
            def make_fillers():
                F = {(st, p): {i: [] for i in range(N_TC)} for st in range(N_ST)
                     for p in range(2)}
                # kv/q production spread over st0
                F[(0, 0)][0] += [lambda: k_chain(0, 1), lambda: v_chain(1)]
                for i, tt in [(1, 2), (2, 3), (3, 4)]:
                    F[(0, 0)][i] += [lambda t=tt: v_chain(t)]
                F[(0, 0)][4] += [lambda: k_chain(0, 2), lambda: v_chain(5)]
                for i, tt in [(5, 6), (6, 7), (7, 8)]:
                    F[(0, 0)][i] += [lambda t=tt: v_chain(t)]
                F[(0, 0)][8] += [lambda: k_chain(0, 3), lambda: v_chain(9)]
                for i, tt in [(9, 10), (10, 11)]:
                    F[(0, 0)][i] += [lambda t=tt: v_chain(t)]
                F[(0, 0)][11] += [lambda: k_chain(1, 0), lambda: v_chain(12)]
                F[(0, 0)][12] += [lambda: q_chain(1, 0), lambda: v_chain(13)]
                F[(0, 0)][13] += [lambda: k_chain(1, 1), lambda: v_chain(14)]
                F[(0, 0)][14] += [lambda: v_chain(15)]
                F[(0, 1)][4] += [lambda: k_chain(1, 2)]
                F[(0, 1)][6] += [lambda: k_chain(1, 3)]
                # q for next s-tile: paired so the scores-psum ring keeps its
                # 2-iteration lookahead (a lone interposed alloc would reduce
                # it to 1 and stall the Scalar engine by ~0.5us per iteration)
                F[(0, 1)][10] += [lambda: q_chain(0, 1)]
                F[(0, 1)][12] += [lambda: q_chain(1, 1)]
                # q for st+1 in the (st,0) loops at i13/i14: past the norm
                # chain's reciprocal (so the DVE finishers don't queue behind
                # it) and adjacent to the outproj singles so the psa ring sees
                # one extra alloc EVERY iteration — scores then always land on
                # fast-freed slots and keep their lookahead
                for st in range(1, N_ST - 1):
                    F[(st, 0)][13] += [lambda s=st: q_chain(0, s + 1)]
                    F[(st, 0)][14] += [lambda s=st: q_chain(1, s + 1)]
                # output projection of s-tile st-1: one double-width (full-E)
                # projection per odd iteration, after cn(st-1,1) is ready
                for st in range(1, N_ST):
                    for ss in range(4):
                        F[(st, 0)][9 + ss] += [
                            lambda s=st - 1, x=ss: outproj(s, x)
                        ]
                return F

            fillers = make_fillers()
            loops = [(st, p) for st in range(N_ST) for p in range(2)]

            def warm_dummies(n):
                # Keep the PE's HAM activity window busy (e.g. through the
                # tail's reciprocal or the preamble's DMA wait) so the clock
                # gate doesn't fall back to 1.2GHz.
                ps = psa.tile([128, 2 * ST], F32, tag="a", name="ps_warm")
                for r in range(n):
                    nc.tensor.matmul(
                        ps[:, :ST], dmy_w[:], dmy_x[:], start=True, stop=True
                    )

            # preamble: warm the PE while the first input DMAs land
            warm_dummies(9)
            k_chain(0, 0)
            q_chain(0, 0)
            v_chain(0)

            carry = []  # closures to inject at the start of the next loop
            for li, (st, p) in enumerate(loops):
                lag = 2 if li == len(loops) - 1 else LAG
                my_fill = fillers[(st, p)]
                for i in range(N_TC):
                    sc = scores(st, p, i)
                    # carried work from the previous loop: ctx tail + den + norm
                    if i < len(carry):
                        carry[i]()
                    for f in my_fill[i]:
                        f()
                    exp_emit(st, p, i, sc)
                    if i >= lag and (i - lag) % 2 == 1:
                        ctx_dr(st, p, (i - lag) // 2)
                # build next carry: finish this loop's ctx, den, then norm
                nxt = []
                for tp in range((N_TC - lag) // 2, N_TC // 2):
                    nxt.append(lambda t=tp, s=st, q=p: ctx_dr(s, q, t))
                nxt.append(lambda s=st, q=p: den_copies(s, q))
                nxt.append(lambda s=st, q=p: norm(s, q))
                carry = nxt

            # tail: flush the last carry items (ctx tail + den of (3,1)),
            # then a quartered normalization so each 128-column block of cn
            # feeds its output projection as soon as its reciprocal lands,
            # with warm-keeper dummies covering the PE through the DVE chain
            for f in carry[:-1]:
                f()
            a3, b3, dAh3, dBh3 = ctx_ps.pop((N_ST - 1, 1))
            rbp3 = psa.tile([128, 2 * ST], F32, tag="a", name="rbp3")
            nc.tensor.matmul(
                rbp3[0:64, :ST], ones1[:], dAh3[:], start=True, stop=True
            )
            nc.tensor.matmul(
                rbp3[64:128, :ST], ones1[:], dBh3[:], start=True, stop=True
            )
            warm_dummies(10)
            rb32_3 = rb32_pool.tile([128, ST], F32, name="rb32_3")
            nc.vector.tensor_copy(rb32_3[:], rbp3[:, :ST])
            rbh3 = rbh_pool.tile([128, ST], FP16, name="rbh3")
            c3 = cn_pool.tile([128, ST], BF16, name="cn3")
            cn[(N_ST - 1, 1)] = c3
            for qq in range(4):
                cs = slice(128 * qq, 128 * (qq + 1))
                with nc.allow_low_precision(reason="fp16 recip of softmax denom"):
                    nc.vector.reciprocal(rbh3[:, cs], rb32_3[:, cs])
                nc.vector.tensor_tensor(
                    c3[0:64, cs], a3[0:64, cs], rbh3[0:64, cs], MULT
                )
                nc.vector.tensor_tensor(
                    c3[64:128, cs], b3[64:128, cs], rbh3[64:128, cs], MULT
                )
                nc.vector.tensor_scalar(
                    c3[:, cs], c3[:, cs], bv_sb[:, 1:2], None, ADD
                )
                outproj(N_ST - 1, qq)
    _split_multi_waits(nc)
    return nc


_NC = None


def _get_nc():
    global _NC
    if _NC is None:
        _NC = build_bass()
    return _NC


def make_in_maps(hidden_states, Wq, bq, Wk, bk, Wv, bv, Wo):
    """Host-side sharding/layout prep. Returns list of 8 per-core input dicts."""
    hs = np.asarray(hidden_states, dtype=np.float32)
    Wq = np.asarray(Wq, dtype=np.float32)
    Wk = np.asarray(Wk, dtype=np.float32)
    Wv = np.asarray(Wv, dtype=np.float32)
    Wo = np.asarray(Wo, dtype=np.float32)
    bq = np.asarray(bq, dtype=np.float32)
    bv = np.asarray(bv, dtype=np.float32)

    # xt host layout: [p, quarter, o, s] flattened -> every xt DMA is one
    # contiguous 8KB-per-partition transfer
    xt = [
        np.ascontiguousarray(
            hs[b].T.reshape(N_EC, 128, 4, ST).transpose(1, 2, 0, 3)
        ).astype(BF16_NP).reshape(-1)
        for b in range(B)
    ]
    in_maps = []
    for c in range(N_CORES):
        b, g = divmod(c, N_CORES // B)
        h0 = HEADS_PER_CORE * g
        hsl = slice(h0, h0 + HEADS_PER_CORE)
        # [H_loc, E, DH] -> [E, H_loc*DH] head-major columns
        def wlay(W):  # [E, EL] -> [p, o, d] flattened (contiguous DMA)
            return np.ascontiguousarray(
                W.reshape(N_EC, 128, EL).transpose(1, 0, 2)
            ).astype(BF16_NP).reshape(-1)

        wq_c = wlay(Wq[hsl].transpose(1, 0, 2).reshape(E, EL))
        wk_c = wlay(Wk[hsl].transpose(1, 0, 2).reshape(E, EL))
        wv_c = wlay(Wv[hsl].transpose(1, 0, 2).reshape(E, EL))
        wo_c = np.ascontiguousarray(
            Wo[EL * g : EL * (g + 1), :].reshape(2, 128, E).transpose(1, 0, 2)
        ).astype(BF16_NP).reshape(-1)
        bq_c = np.ascontiguousarray(bq[hsl].reshape(EL).reshape(2, 128).T)
        bv_c = np.ascontiguousarray(bv[hsl].reshape(EL).reshape(2, 128).T)
        in_maps.append(
            {
                "xt": xt[b],
                "wq": wq_c,
                "wk": wk_c,
                "wv": wv_c,
                "wo": wo_c,
                "bq2": bq_c,
                "bv2": bv_c,
            }
        )
    return in_maps


def kernel(hidden_states, mask, Wq, bq, Wk, bk, Wv, bv, Wo, bo, **run_kwargs):
    """Full-input entry point. mask is all-ones per the problem spec (ignored)."""
    nc = _get_nc()
    in_maps = make_in_maps(hidden_states, Wq, bq, Wk, bk, Wv, bv, Wo)
    res = run_bass_kernel_spmd(nc, in_maps, core_ids=list(range(N_CORES)), **run_kwargs)
    bo = np.asarray(bo, dtype=np.float32)
    out = np.zeros((B, S, E), dtype=np.float32)
    for c in range(N_CORES):
        out[c // (N_CORES // B)] += res.results[c]["out"].astype(np.float32)
    out += bo
    kernel.last_results = res
    return out


# revision 16
# speedup vs baseline: 1.0633x; 1.0633x over previous
"""Multi-head attention (B=2, S=2048, E=1024, H=16) on 8 Trainium2 NeuronCores.

Sharding: core c handles batch b=c//4 and head group g=c%4 (4 heads each).
hidden_states[b] is replicated to the 4 cores of batch b (pre-transposed and
cast to bf16 on host so the contraction dim E lands on SBUF partitions with
plain contiguous DMAs). Each core computes q/k/v projections for its heads,
transposed-layout attention, and a partial output projection over its 256
E-dims. The host sums the 4 partials per batch and adds bo.

Schedule: a single software-pipelined stream. Projections (k/q/v chains) are
injected into the first s-tile's attention iterations so the exp stream on the
Scalar engine starts ~8us in and the PE never idles long enough for the HAM
clock gate to re-throttle. The softmax denominator is fused into the ctx
matmul: head A's stationary is [vA(64) | ones | 0*63] (ctx at psum partitions
0:64, denom at 64), head B's is [ones | 0*63 | vB(64)] (denom at partition 0,
ctx at 64:128) so all downstream DVE ops are partition-aligned and the output
projection keeps C=128 contraction. Normalization: denominators are copied to
fp16, broadcast over 64 partitions with a tiny ones-matmul, reciprocal'd once
per [128,512] tile, multiplied into cn, bias-added.

Bias handling: softmax over t is invariant to per-query constants, so the
k-bias drops out and the q-bias folds into q. The v-bias is a post-softmax
additive constant (softmax rows sum to 1). bo is added on host.
"""

import sys

if "/opt/trn_rl_repo" not in sys.path:
    sys.path.insert(0, "/opt/trn_rl_repo")

import numpy as np
import ml_dtypes

import concourse.bass as bass
import concourse.tile as tile
from concourse import mybir
from concourse.bass_utils import run_bass_kernel_spmd
from concourse.vector_clock import ScopedClock

B, S, E, H = 2, 2048, 1024, 16
DH = E // H  # 64
N_CORES = 8
HEADS_PER_CORE = 4  # 2 pairs
EL = HEADS_PER_CORE * DH  # 256 local E-dims per core

F32 = mybir.dt.float32
BF16 = mybir.dt.bfloat16
FP16 = mybir.dt.float16
FP8 = mybir.dt.float8e4
BF16_NP = ml_dtypes.bfloat16

ST = 512  # s_tile width
N_ST = S // ST  # 4
N_TC = S // 128  # 16 t-chunks
N_EC = E // 128  # 8 e-chunks
LAG = 4  # ctx matmuls trail scores/exp by this many iterations


def _patch_tail_drain():
    """walrus CoreV3 setupSyncWait allows only 1 sem wait on an SP Drain; Tile's
    kernel-tail drain carries one wait per live processor. Split the waits
    across consecutive drains (mutating via nc.inst_map, whose objects are what
    to_json_bytes serializes)."""
    if getattr(tile.TileContext, "_drain_patched", False):
        return

    def _drain_and_barrier(self, tick_clock, wait_clock):
        nc = self.nc
        drain_inst = nc.sync.drain()
        wait_clock.add_sem_waits(
            drain_inst.ins, ScopedClock({None: tick_clock.global_clock})
        )
        inst = nc.inst_map[drain_inst.ins.name]
        w = list(inst.sync_info.on_wait) if inst.sync_info else []
        if len(w) > 1:
            si = inst.sync_info
            si.on_wait = w[:1]
            inst.sync_info = si
            for i in range(1, len(w)):
                d2 = nc.sync.drain()
                i2 = nc.inst_map[d2.ins.name]
                si2 = i2.sync_info or mybir.SyncInfo(on_wait=[], on_update=[])
                si2.on_wait = [w[i]]
                i2.sync_info = si2
        nc.all_engine_barrier()
        assert self.sems is not None
        popped = nc._tile_sem_poison_stack.pop()
        assert popped is self._sem_poison
        nc.clear_and_free_semaphores(list(self.sems.allocated().values()))
        nc.all_engine_barrier()

    tile.TileContext._drain_and_barrier = _drain_and_barrier
    tile.TileContext._drain_patched = True


def _split_multi_waits(nc):
    """The walrus build in this environment accepts only ONE sem-wait command
    per instruction, but Tile's wait-assignment attaches several. Hoist excess
    waits onto dedicated same-engine no-op carrier instructions inserted
    immediately before the owner (same engine-stream position, identical
    semantics)."""
    f = nc.m.functions[0]
    blocks = list(f.blocks)
    carriers: dict[str, list] = {}
    created = set()
    for blk in blocks:
        for inst in blk.instructions:
            if inst.sync_info and len(inst.sync_info.on_wait) > 1:
                w = list(inst.sync_info.on_wait)
                cs = []
                for wx in w[:-1]:
                    # engine nop() appends to nc.cur_bb; it is re-homed below
                    nop = nc.engines[inst.engine].nop(nofuse=True).ins
                    nop.sync_info = mybir.SyncInfo(on_wait=[wx], on_update=[])
                    cs.append(nop)
                    created.add(nop.name)
                si = inst.sync_info
                si.on_wait = [w[-1]]
                inst.sync_info = si
                carriers[inst.name] = cs
    if not carriers:
        return
    for blk in blocks:
        rebuilt = []
        for i in blk.instructions:
            if i.name in created:
                continue
            rebuilt.extend(carriers.get(i.name, ()))
            rebuilt.append(i)
        blk.instructions = rebuilt


def build_bass():
    """Build the per-core Bass program (identical on all 8 cores)."""
    _patch_tail_drain()
    nc = bass.Bass("TRN2", target_bir_lowering=False, debug=False)

    xt_d = nc.dram_tensor("xt", [E * S], BF16, kind="ExternalInput").ap()
    wq_d = nc.dram_tensor("wq", [E * EL], BF16, kind="ExternalInput").ap()
    wk_d = nc.dram_tensor("wk", [E * EL], BF16, kind="ExternalInput").ap()
    wv_d = nc.dram_tensor("wv", [E * EL], BF16, kind="ExternalInput").ap()
    wo_d = nc.dram_tensor("wo", [EL * E], BF16, kind="ExternalInput").ap()
    bq_d = nc.dram_tensor("bq2", [128, 2], F32, kind="ExternalInput").ap()
    bv_d = nc.dram_tensor("bv2", [128, 2], F32, kind="ExternalInput").ap()
    out_d = nc.dram_tensor("out", [S, E], BF16, kind="ExternalOutput").ap()

    EXP = mybir.ActivationFunctionType.Exp
    ADD = mybir.AluOpType.add
    MULT = mybir.AluOpType.mult

    with tile.TileContext(nc) as tc:
        with (
            tc.tile_pool(name="const", bufs=1) as const_pool,
            tc.tile_pool(name="xw", bufs=1) as xw_pool,
            tc.tile_pool(name="qkv", bufs=1) as qkv_pool,
            tc.tile_pool(name="exs", bufs=5) as ex_pool,
            tc.tile_pool(name="cns", bufs=3) as cn_pool,
            tc.tile_pool(name="rb32", bufs=2) as rb32_pool,
            tc.tile_pool(name="rbh", bufs=2) as rbh_pool,
            tc.tile_pool(name="dh", bufs=4) as dh_pool,
            tc.tile_pool(name="outs", bufs=3) as out_pool,
            tc.tile_pool(name="psa", bufs=2, space="PSUM") as psa,
            tc.tile_pool(name="psc", bufs=4, space="PSUM") as psc,
        ):
            # ---- constants and weights
            ones1 = const_pool.tile([1, 64], FP16)
            nc.vector.memset(ones1[:], 1.0)
            # Full-array dummy operands: HAM gauges PE *array activity*, so
            # warmers must light up all 128x128 cells (a [1,64] ones matmul
            # reads as idle and the clock gate stays at 1.2GHz).
            dmy_w = const_pool.tile([128, 128], BF16, name="dmy_w")
            nc.vector.memset(dmy_w[:], 0.0)
            dmy_x = const_pool.tile([128, ST], BF16, name="dmy_x")
            nc.vector.memset(dmy_x[:], 0.0)

            bq_sb = const_pool.tile([128, 2], F32)
            nc.sync.dma_start(bq_sb[:], bq_d[:])
            bv_sb = const_pool.tile([128, 2], F32)
            nc.sync.dma_start(bv_sb[:], bv_d[:])

            # weights arrive host-pre-transposed to [128, o, d] so every DMA
            # is contiguous 4KB-per-partition; wk/wq + the first xt quarter are
            # emitted first so the k/q chains start within a few us.
            # per-ec interleaved wk/xt-q0 transfers: the k(0,0) chain's ec-th
            # matmul only depends on the ec-th slices (subtile deps), so the
            # PE starts ~2us after the first pair lands instead of waiting for
            # every input
            wk_sb = xw_pool.tile([128, N_EC, EL], BF16)
            wq_sb = xw_pool.tile([128, N_EC, EL], BF16)
            wv_sb = xw_pool.tile([128, N_EC, EL], BF16)
            xt_sb = xw_pool.tile([128, N_EC, S], BF16)
            wk_r = wk_d.rearrange("(p o d) -> p o d", p=128, o=N_EC)
            wq_r = wq_d.rearrange("(p o d) -> p o d", p=128, o=N_EC)
            wv_r = wv_d.rearrange("(p o d) -> p o d", p=128, o=N_EC)
            xt_q = xt_d.rearrange("(p q o s) -> p q o s", p=128, q=4, o=N_EC)
            for h in range(2):
                e0, e1 = 4 * h, 4 * (h + 1)
                nc.sync.dma_start(wk_sb[:, e0:e1, :], wk_r[:, e0:e1])
                nc.sync.dma_start(xt_sb[:, e0:e1, 0:512], xt_q[:, 0, e0:e1])
            nc.sync.dma_start(wq_sb[:], wq_r)
            nc.sync.dma_start(wv_sb[:], wv_r)
            for q4 in range(1, 4):
                nc.sync.dma_start(
                    xt_sb[:, :, 512 * q4 : 512 * (q4 + 1)], xt_q[:, q4]
                )
            wo_sb = xw_pool.tile([128, 2, E], BF16)
            nc.sync.dma_start(wo_sb[:], wo_d.rearrange("(p o n) -> p o n", p=128, o=2))

            # ---- persistent SBUF tensors
            qT = [qkv_pool.tile([128, S], BF16, name=f"qT{p}") for p in range(2)]
            kT = [qkv_pool.tile([128, S], BF16, name=f"kT{p}") for p in range(2)]
            # v65: per (t-chunk, head) a 128-col stationary.
            #   even head (A): [v(64) | ones | 0*63]  -> ctx @ psum 0:64, den @ 64
            #   odd head (B):  [ones | 0*63 | v(64)]  -> den @ psum 0, ctx @ 64:128
            v65 = qkv_pool.tile([128, N_TC, 4, 128], FP8)
            nc.vector.memset(v65[:, :, 0::2, 64:65], 1.0)
            nc.vector.memset(v65[:, :, 0::2, 65:128], 0.0)
            nc.vector.memset(v65[:, :, 1::2, 0:1], 1.0)
            nc.vector.memset(v65[:, :, 1::2, 1:64], 0.0)

            cn = {}  # (st, p) -> cn tile

            # ---------------- emission closures ----------------
            def k_chain(p, kt):
                ps = psa.tile([128, 2 * ST], F32, tag="a", name="ps_k")
                for ec in range(N_EC):
                    nc.tensor.matmul(
                        ps[:, :ST],
                        wk_sb[:, ec, 128 * p : 128 * (p + 1)],
                        xt_sb[:, ec, ST * kt : ST * (kt + 1)],
                        start=(ec == 0),
                        stop=(ec == N_EC - 1),
                    )
                nc.vector.tensor_copy(kT[p][:, ST * kt : ST * (kt + 1)], ps[:, :ST])

            def q_chain(p, st):
                ps = psa.tile([128, 2 * ST], F32, tag="a", name="ps_q")
                for ec in range(N_EC):
                    nc.tensor.matmul(
                        ps[:, :ST],
                        wq_sb[:, ec, 128 * p : 128 * (p + 1)],
                        xt_sb[:, ec, ST * st : ST * (st + 1)],
                        start=(ec == 0),
                        stop=(ec == N_EC - 1),
                    )
                nc.vector.tensor_scalar(
                    qT[p][:, ST * st : ST * (st + 1)],
                    ps[:, :ST],
                    bq_sb[:, p : p + 1],
                    None,
                    ADD,
                )

            def v_chain(tt):
                ps = psa.tile([128, 2 * ST], F32, tag="a", name="ps_v")
                for ec in range(N_EC):
                    nc.tensor.matmul(
                        ps[:, :EL],
                        xt_sb[:, ec, 128 * tt : 128 * (tt + 1)],
                        wv_sb[:, ec, :],
                        start=(ec == 0),
                        stop=(ec == N_EC - 1),
                    )
                vsrc = ps[:, :EL].rearrange("p (h d) -> p h d", h=4)
                # even heads -> cols 0:64, odd heads -> cols 64:128
                nc.vector.tensor_copy(v65[:, tt, 0::2, 0:64], vsrc[:, 0::2, :])
                nc.vector.tensor_copy(v65[:, tt, 1::2, 64:128], vsrc[:, 1::2, :])

            ctx_ps = {}  # (st, p) -> (ctxA tile, ctxB tile)
            ex_tiles = {}  # (st, p, tc) -> ex tile (deleted after use)

            def scores(st, p, tcc):
                sc = psa.tile([128, 2 * ST], F32, tag="a", name="sc")
                nc.tensor.matmul(
                    sc[:, :ST],
                    kT[p][0:64, 128 * tcc : 128 * (tcc + 1)],
                    qT[p][0:64, ST * st : ST * (st + 1)],
                    start=True,
                    stop=True,
                )
                nc.tensor.matmul(
                    sc[:, ST:],
                    kT[p][64:128, 128 * tcc : 128 * (tcc + 1)],
                    qT[p][64:128, ST * st : ST * (st + 1)],
                    start=True,
                    stop=True,
                )
                return sc

            def exp_emit(st, p, tcc, sc):
                # fp8 softmax weights, written pair-interleaved for DoubleRow
                if tcc % 2 == 0:
                    ex = ex_pool.tile([128, 2, 2 * ST], FP8, name="ex")
                    ex_tiles[(st, p, tcc // 2)] = ex
                else:
                    ex = ex_tiles[(st, p, tcc // 2)]
                nc.scalar.activation(ex[:, tcc % 2, :], sc[:], EXP, scale=0.125)

            def ctx_dr(st, p, tp):
                # one fp8 DoubleRow matmul per head covers two t-chunks
                if (st, p) not in ctx_ps:
                    a = psc.tile([128, ST], F32, tag="c", name="ctxA")
                    b = psc.tile([128, ST], F32, tag="c", name="ctxB")
                    ctx_ps[(st, p)] = (a, b)
                a, b = ctx_ps[(st, p)]
                ex = ex_tiles.pop((st, p, tp))
                first, last = tp == 0, tp == N_TC // 2 - 1
                nc.tensor.matmul(
                    a[:],
                    v65[:, 2 * tp : 2 * tp + 2, 2 * p, :],
                    ex[:, :, :ST],
                    start=first,
                    stop=last,
                    perf_mode=mybir.MatmulPerfMode.DoubleRow,
                )
                nc.tensor.matmul(
                    b[:],
                    v65[:, 2 * tp : 2 * tp + 2, 2 * p + 1, :],
                    ex[:, :, ST:],
                    start=first,
                    stop=last,
                    perf_mode=mybir.MatmulPerfMode.DoubleRow,
                )

            def den_copies(st, p):
                a, b = ctx_ps[(st, p)]
                dAh = dh_pool.tile([1, ST], FP16, tag="dh", name="dAh")
                nc.vector.tensor_copy(dAh[:], a[64:65, :])
                dBh = dh_pool.tile([1, ST], FP16, tag="dh", name="dBh")
                nc.vector.tensor_copy(dBh[:], b[0:1, :])
                ctx_ps[(st, p)] = (a, b, dAh, dBh)

            def norm(st, p):
                a, b, dAh, dBh = ctx_ps.pop((st, p))
                rbp = psa.tile([128, 2 * ST], F32, tag="a", name="rbp")
                nc.tensor.matmul(
                    rbp[0:64, :ST], ones1[:], dAh[:], start=True, stop=True
                )
                nc.tensor.matmul(
                    rbp[64:128, :ST], ones1[:], dBh[:], start=True, stop=True
                )
                rb32 = rb32_pool.tile([128, ST], F32, name="rb32")
                nc.vector.tensor_copy(rb32[:], rbp[:, :ST])
                rbh = rbh_pool.tile([128, ST], FP16, name="rbh")
                with nc.allow_low_precision(reason="fp16 recip of softmax denom"):
                    nc.vector.reciprocal(rbh[:], rb32[:])
                c = cn_pool.tile([128, ST], BF16, name="cn")
                nc.vector.tensor_tensor(c[0:64, :], a[0:64, :], rbh[0:64, :], MULT)
                nc.vector.tensor_tensor(
                    c[64:128, :], b[64:128, :], rbh[64:128, :], MULT
                )
                nc.vector.tensor_scalar(c[:], c[:], bv_sb[:, p : p + 1], None, ADD)
                cn[(st, p)] = c

            def outproj(st, ss):
                # full-E projection of one 128-row block: 4 F=512 matmuls into
                # the two bank-halves of a single ring slot (one alloc, so the
                # scores ring keeps its lookahead), one copy, one DMA
                ps = psa.tile([128, 2 * ST], F32, tag="a", name="ps_o")
                for nt in range(2):
                    for p in range(2):
                        nc.tensor.matmul(
                            ps[:, ST * nt : ST * (nt + 1)],
                            cn[(st, p)][:, 128 * ss : 128 * (ss + 1)],
                            wo_sb[:, p, ST * nt : ST * (nt + 1)],
                            start=(p == 0),
                            stop=(p == 1),
                        )
                ob = out_pool.tile([128, 2 * ST], BF16, name="ob")
                nc.vector.tensor_copy(ob[:], ps[:])
                srow = ST * st + 128 * ss
                nc.sync.dma_start(out_d[srow : srow + 128, :], ob[:])

            # ---------------- schedule ----------------
            # Injections per loop (st, p), keyed by iteration index.
            def make_fillers():
                F = {(st, p): {i: [] for i in range(N_TC)} for st in range(N_ST)
                     for p in range(2)}
                # kv/q production spread over st0
                F[(0, 0)][0] += [lambda: k_chain(0, 1), lambda: v_chain(1)]
                for i, tt in [(1, 2), (2, 3), (3, 4)]:
                    F[(0, 0)][i] += [lambda t=tt: v_chain(t)]
                F[(0, 0)][4] += [lambda: k_chain(0, 2), lambda: v_chain(5)]
                for i, tt in [(5, 6), (6, 7), (7, 8)]:
                    F[(0, 0)][i] += [lambda t=tt: v_chain(t)]
                F[(0, 0)][8] += [lambda: k_chain(0, 3), lambda: v_chain(9)]
                for i, tt in [(9, 10), (10, 11)]:
                    F[(0, 0)][i] += [lambda t=tt: v_chain(t)]
                F[(0, 0)][11] += [lambda: k_chain(1, 0), lambda: v_chain(12)]
                F[(0, 0)][12] += [lambda: q_chain(1, 0), lambda: v_chain(13)]
                F[(0, 0)][13] += [lambda: k_chain(1, 1), lambda: v_chain(14)]
                F[(0, 0)][14] += [lambda: v_chain(15)]
                F[(0, 1)][0] += [lambda: k_chain(1, 2)]
                F[(0, 1)][2] += [lambda: k_chain(1, 3)]
                # q for next s-tile: paired so the scores-psum ring keeps its
                # 2-iteration lookahead (a lone interposed alloc would reduce
                # it to 1 and stall the Scalar engine by ~0.5us per iteration)
                for st in range(N_ST - 1):
                    F[(st, 1)][10] += [lambda s=st: q_chain(0, s + 1)]
                    F[(st, 1)][12] += [lambda s=st: q_chain(1, s + 1)]
                # output projection of s-tile st-1: one double-width (full-E)
                # projection per odd iteration, after cn(st-1,1) is ready
                for st in range(1, N_ST):
                    for ss in range(4):
                        F[(st, 0)][9 + 2 * ss] += [
                            lambda s=st - 1, x=ss: outproj(s, x)
                        ]
                return F

            fillers = make_fillers()
            loops = [(st, p) for st in range(N_ST) for p in range(2)]

            def warm_dummies(n):
                # Keep the PE's HAM activity window busy (e.g. through the
                # tail's reciprocal or the preamble's DMA wait) so the clock
                # gate doesn't fall back to 1.2GHz.
                ps = psa.tile([128, 2 * ST], F32, tag="a", name="ps_warm")
                for r in range(n):
                    nc.tensor.matmul(
                        ps[:, :ST], dmy_w[:], dmy_x[:], start=True, stop=True
                    )

            # preamble: warm the PE while the first input DMAs land
            warm_dummies(6)
            k_chain(0, 0)
            q_chain(0, 0)
            v_chain(0)

            carry = []  # closures to inject at the start of the next loop
            for li, (st, p) in enumerate(loops):
                lag = 2 if li == len(loops) - 1 else LAG
                my_fill = fillers[(st, p)]
                for i in range(N_TC):
                    sc = scores(st, p, i)
                    # carried work from the previous loop: ctx tail + den + norm
                    if i < len(carry):
                        carry[i]()
                    for f in my_fill[i]:
                        f()
                    exp_emit(st, p, i, sc)
                    if i >= lag and (i - lag) % 2 == 1:
                        ctx_dr(st, p, (i - lag) // 2)
                # build next carry: finish this loop's ctx, den, then norm
                nxt = []
                for tp in range((N_TC - lag) // 2, N_TC // 2):
                    nxt.append(lambda t=tp, s=st, q=p: ctx_dr(s, q, t))
                nxt.append(lambda s=st, q=p: den_copies(s, q))
                nxt.append(lambda s=st, q=p: norm(s, q))
                carry = nxt

            # tail: flush the last carry (ctx tail, den, norm of (3,1)) with
            # warm-keeper dummies over the reciprocal window, then the final
            # output projection
            for f in carry:
                f()
            warm_dummies(10)
            for ss in range(4):
                outproj(N_ST - 1, ss)
    _split_multi_waits(nc)
    return nc


_NC = None


def _get_nc():
    global _NC
    if _NC is None:
        _NC = build_bass()
    return _NC


def make_in_maps(hidden_states, Wq, bq, Wk, bk, Wv, bv, Wo):
    """Host-side sharding/layout prep. Returns list of 8 per-core input dicts."""
    hs = np.asarray(hidden_states, dtype=np.float32)
    Wq = np.asarray(Wq, dtype=np.float32)
    Wk = np.asarray(Wk, dtype=np.float32)
    Wv = np.asarray(Wv, dtype=np.float32)
    Wo = np.asarray(Wo, dtype=np.float32)
    bq = np.asarray(bq, dtype=np.float32)
    bv = np.asarray(bv, dtype=np.float32)

    # xt host layout: [p, quarter, o, s] flattened -> every xt DMA is one
    # contiguous 8KB-per-partition transfer
    xt = [
        np.ascontiguousarray(
            hs[b].T.reshape(N_EC, 128, 4, ST).transpose(1, 2, 0, 3)
        ).astype(BF16_NP).reshape(-1)
        for b in range(B)
    ]
    in_maps = []
    for c in range(N_CORES):
        b, g = divmod(c, N_CORES // B)
        h0 = HEADS_PER_CORE * g
        hsl = slice(h0, h0 + HEADS_PER_CORE)
        # [H_loc, E, DH] -> [E, H_loc*DH] head-major columns
        def wlay(W):  # [E, EL] -> [p, o, d] flattened (contiguous DMA)
            return np.ascontiguousarray(
                W.reshape(N_EC, 128, EL).transpose(1, 0, 2)
            ).astype(BF16_NP).reshape(-1)

        wq_c = wlay(Wq[hsl].transpose(1, 0, 2).reshape(E, EL))
        wk_c = wlay(Wk[hsl].transpose(1, 0, 2).reshape(E, EL))
        wv_c = wlay(Wv[hsl].transpose(1, 0, 2).reshape(E, EL))
        wo_c = np.ascontiguousarray(
            Wo[EL * g : EL * (g + 1), :].reshape(2, 128, E).transpose(1, 0, 2)
        ).astype(BF16_NP).reshape(-1)
        bq_c = np.ascontiguousarray(bq[hsl].reshape(EL).reshape(2, 128).T)
        bv_c = np.ascontiguousarray(bv[hsl].reshape(EL).reshape(2, 128).T)
        in_maps.append(
            {
                "xt": xt[b],
                "wq": wq_c,
                "wk": wk_c,
                "wv": wv_c,
                "wo": wo_c,
                "bq2": bq_c,
                "bv2": bv_c,
            }
        )
    return in_maps


def kernel(hidden_states, mask, Wq, bq, Wk, bk, Wv, bv, Wo, bo, **run_kwargs):
    """Full-input entry point. mask is all-ones per the problem spec (ignored)."""
    nc = _get_nc()
    in_maps = make_in_maps(hidden_states, Wq, bq, Wk, bk, Wv, bv, Wo)
    res = run_bass_kernel_spmd(nc, in_maps, core_ids=list(range(N_CORES)), **run_kwargs)
    bo = np.asarray(bo, dtype=np.float32)
    out = np.zeros((B, S, E), dtype=np.float32)
    for c in range(N_CORES):
        out[c // (N_CORES // B)] += res.results[c]["out"].astype(np.float32)
    out += bo
    kernel.last_results = res
    return out
